# revision 15
# baseline (speedup 1.0000x reference)
"""Trainium2 Bass kernel: unnormalized single-head attention block.

Computes, for x [4, 4096, 1024] and w_q/w_k/w_v/w_o [1024, 1024] (all fp32):
    q = x @ w_q ; k = x @ w_k ; v = x @ w_v
    scores = q @ k.T            (no softmax)
    out = (scores @ v) @ w_o

No softmax, so the chain collapses to
    out_b = x_b @ [ w_q @ w_k.T @ (x_b.T @ x_b) @ w_v @ w_o ]
i.e. a Gram matrix G_b = x_b.T @ x_b plus a short chain of 1024^3 matmuls
(~90 GFLOP total instead of ~412 GFLOP).

Sharding: 8 NeuronCores = (4 batches) x (2 sequence halves), core = 2b + h.
Pair groups [[0,1],[2,3],[4,5],[6,7]] exchange G and M.

Measured system constants this schedule is built around:
  - the CC engine has a one-time ~40us startup before its first collective
    executes (and activates at ~20us); a tiny dummy pair-ReduceScatter
    prepays it so the real collectives start the moment they're triggered;
  - a pair collective costs ~6us + ~21us/MB of peer payload;
  - the PE sustains 1 matmul column/cycle at 2.4 GHz but re-throttles to
    1.2 GHz after long idles, so the schedule keeps it continuously fed.

Schedule (PE order):
  1. ~16 dummy matmuls warm the HAM clock gate while the x DMAs stream.
  2. G upper triangle only (G is symmetric): per 128-row tile jt, cols
     >= 128*jt, packed into a 1.125 MB triangle -> pairwise AllReduce.
     wqT tile loads are interleaved between the per-row staging stores on
     the scalar queue so neither blocks the other.
  3. C = w_v @ w_o[:, own 640 cols]: at the 640/384 R/M column split each
     core only consumes its own 640-column window of C, so the host
     pre-rolls/slices wo and C is computed at 640 width.
  4. AT = w_k @ w_q.T (27us, needed in full for M) fills the rest of the
     AllReduce window, hiding it completely.
  5. Post-AR: load the summed triangle, rebuild the 28 lower lhsT tiles
     with PE transposes (row 7 of R needs none, so it is emitted first).
  6. R = G @ C and M = AT.T @ R for rotated cols [0:640) -- "rotated" col
     e is natural col (e + 512h) mod 1024, realized purely by the host's
     wo rolling, which keeps the SPMD program rank-free. Both pair
     members lack exactly the peer's rotated cols [128:512), so that
     slice is exchanged via a masked pair ReduceScatter (own slot zeroed)
     while out = x_own @ M[:, 0:640) runs on the PE (34us of cover for a
     ~22us exchange).
  7. out psum is stored bf16 (host casts back to fp32), stores alternate
     scalar/sync queues; the host un-rotates odd cores' output columns.

Device math is bf16 with fp32 PSUM accumulation (rel err ~5.9e-3 vs fp32
reference). The host ships bf16 tensors directly so no on-device layout
changes or casts are needed.
"""

import contextlib
import ctypes
import os
import sys
import types

import numpy as np

B = 4
T = 4096
D = 1024
H = T // 2          # rows per core
P = 128             # SBUF partitions
NCORES = 8
DT = D // P         # 8 tiles along any 1024 dim
TT = H // P         # 16 own-half t-tiles
FREE = 512          # matmul moving free dim / PSUM bank width (fp32)
KC = D // FREE      # 2 free-dim chunks of 512 along a 1024 dim
PAIRS = [[0, 1], [2, 3], [4, 5], [6, 7]]
WARMUP = 16    # dummy matmuls to warm the HAM clock gate during the first DMAs
FILLER = 8     # post-AT dummy matmuls so an AR overrun can't re-throttle the PE
OWN = 640      # rotated M/R columns computed locally (peer computes the rest)
SEND0 = OWN - 512   # sent M slice [128:512) -- rank-free by pair symmetry
SENDW = D - OWN     # 384

_STATE = {}
LAST_RESULTS = None


def _install_axon_ntff_shim():
    """bass_utils(trace=True) under axon imports antenv.axon_hooks, which the
    agent image lacks. Provide the documented ctypes equivalent so tracing
    works; degrades to hook=None when the .so has no profile symbols."""
    try:
        import antenv.axon_hooks  # noqa: F401
        return
    except ImportError:
        pass

    so_path = "/opt/axon/libaxon_pjrt.so"

    def _make_hook():
        try:
            lib = ctypes.CDLL(so_path)
        except OSError:
            return None
        if not hasattr(lib, "axon_start_nrt_profile"):
            return None
        lib.axon_start_nrt_profile.argtypes = [
            ctypes.POINTER(ctypes.c_int64),
            ctypes.c_size_t,
        ]
        lib.axon_start_nrt_profile.restype = ctypes.c_int64
        lib.axon_stop_nrt_profile.argtypes = [ctypes.c_char_p]
        lib.axon_stop_nrt_profile.restype = ctypes.c_int64

        @contextlib.contextmanager
        def _hook(output_dir, device_ids):
            import jax

            jax.devices()
            if device_ids:
                ids = (ctypes.c_int64 * len(device_ids))(*device_ids)
                rc = lib.axon_start_nrt_profile(ids, len(device_ids))
            else:
                rc = lib.axon_start_nrt_profile(None, 0)
            if rc != 0:
                raise RuntimeError(f"axon_start_nrt_profile rc={rc}")
            try:
                yield
            finally:
                n = lib.axon_stop_nrt_profile(str(output_dir).encode())
                print(f"profile: {n} file(s) written to {output_dir}", file=sys.stderr)

        return _hook

    mod = types.ModuleType("antenv.axon_hooks")
    mod.get_axon_ntff_profile_hook = _make_hook
    mod.set_axon_ntff_profile_hook = lambda h: None
    sys.modules["antenv.axon_hooks"] = mod


def _trace_kernel(tc, xn, xt, wqT, wkT, wvT, woh, mask, out):
    import concourse.mybir as mybir
    from concourse.bass import ts

    nc = tc.nc
    f32 = mybir.dt.float32
    bf16 = mybir.dt.bfloat16

    with contextlib.ExitStack() as top:
        ps_pool = top.enter_context(tc.tile_pool(name="ps", bufs=8, space="PSUM"))
        dram_pool = top.enter_context(tc.tile_pool(name="cdram", bufs=2, space="DRAM"))
        at_pool = top.enter_context(tc.tile_pool(name="at", bufs=DT))
        c_pool = top.enter_context(tc.tile_pool(name="c", bufs=DT))
        xt_pool = top.enter_context(tc.tile_pool(name="xt", bufs=DT))

        # G triangle: row jt contributes cols >= jt*128, packed into a
        # 1.125 MB strip exchanged with a pairwise AllReduce.
        TRI_OFF = [0] * DT
        for r in range(1, DT):
            TRI_OFF[r] = TRI_OFF[r - 1] + (DT - (r - 1)) * P
        TRI_W = TRI_OFF[-1] + P  # 4608
        gsrc = dram_pool.tile([P, TRI_W], bf16, name="gsrc", tag="gsrc")
        gsum_tri = dram_pool.tile([P, TRI_W], bf16, name="gsumt", tag="gsum")

        # CC-engine warmup dummy collective staging.
        fsrc = dram_pool.tile([2, P, 4], bf16, name="fsrc", tag="flag")
        fsum = dram_pool.tile([P, 4], bf16, name="fsum", tag="flag")

        from concourse import masks

        id_pool = top.enter_context(tc.tile_pool(name="idp", bufs=2))
        wu_pool = top.enter_context(tc.tile_pool(name="wu", bufs=1))
        wu = wu_pool.tile([P, FREE], bf16, name="wu", tag="wu")
        nc.vector.memset(wu[:], 0.0)
        nc.sync.dma_start(out=fsrc[0], in_=wu[:, 0:4])
        nc.sync.dma_start(out=fsrc[1], in_=wu[:, 0:4])
        # Dummy collective: prepays the CC engine's one-time ~40us startup
        # while the input DMAs stream, so the real G-AR starts instantly.
        nc.gpsimd.collective_compute(
            "ReduceScatter",
            mybir.AluOpType.add,
            replica_groups=PAIRS,
            ins=[fsrc.opt()],
            outs=[fsum.opt()],
        )

        ident = id_pool.tile([P, P], bf16, name="ident", tag="id")
        masks.make_identity(nc, ident[:])

        # Pair-position mask for the M-slice exchange (own slot zeroed).
        mb = id_pool.tile([P, 2], f32, name="mb", tag="mb")
        nc.sync.dma_start(out=mb[:], in_=mask)
        mstage = dram_pool.tile([2, DT, P, SENDW], bf16, name="mstage", tag="mst")
        mpeer = dram_pool.tile([DT, P, SENDW], bf16, name="mpeer", tag="mpr")

        wps = ps_pool.tile([P, FREE], f32, name="wps", tag="ps")
        for _ in range(WARMUP):
            nc.tensor.matmul(wps[:], wu[:, :P], wu[:], start=True, stop=True)

        with contextlib.ExitStack() as setup:
            xn_pool = setup.enter_context(tc.tile_pool(name="xn", bufs=TT))
            w_pool = setup.enter_context(tc.tile_pool(name="w", bufs=4 * DT))

            # x tiles: the only DMAs G needs, split over all three queues.
            xns = []
            for t in range(TT):
                xv = xn_pool.tile([P, D], bf16, name=f"xn{t}", tag="xn")
                eng = (nc.sync, nc.scalar, nc.gpsimd)[t % 3]
                eng.dma_start(out=xv[:], in_=xn[ts(t, P), :])
                xns.append(xv)

            def w_tile(tag, i, width):
                return w_pool.tile([P, width], bf16, name=f"{tag}{i}", tag="w")

            # The opening is HBM-bound: only x may stream during G, then
            # wk/wq (for AT, right after G) behind it on sync; wv/wo (for
            # C, last PE phase before R) arrive later on gpsimd/scalar.
            wk_t = []
            for i in range(DT):
                wt = w_tile("wk", i, D)
                nc.sync.dma_start(out=wt[:], in_=wkT[ts(i, P), :])
                wk_t.append(wt)
            wq_t = []
            for i in range(DT):
                wt = w_tile("wq", i, D)
                nc.sync.dma_start(out=wt[:], in_=wqT[ts(i, P), :])
                wq_t.append(wt)

            # --- own-half Gram matrix G[j,k] = sum_t x[t,j] x[t,k] ---
            # Upper triangle only; lower tiles are rebuilt after the AR.
            # wqT tile loads are interleaved between the per-row staging
            # stores so the scalar queue serves both without blocking.
            gown_pool = setup.enter_context(tc.tile_pool(name="gown", bufs=DT))
            gown = [
                gown_pool.tile([P, D], bf16, name=f"go{j}", tag="gown")
                for j in range(DT)
            ]
            for jt in range(DT):
                off = jt * P
                while off < D:
                    w = min(FREE, D - off)
                    psum = ps_pool.tile([P, w], f32, name="psg", tag="ps")
                    for t in range(TT):
                        nc.tensor.matmul(
                            psum[:],
                            xns[t][:, ts(jt, P)],
                            xns[t][:, off : off + w],
                            start=(t == 0),
                            stop=(t == TT - 1),
                        )
                    nc.vector.tensor_copy(gown[jt][:, off : off + w], psum[:])
                    off += w
                rw = (DT - jt) * P
                sl = slice(TRI_OFF[jt], TRI_OFF[jt] + rw)
                nc.scalar.dma_start(out=gsrc[:, sl], in_=gown[jt][:, jt * P :])
                if jt == DT - 1:
                    nc.gpsimd.collective_compute(
                        "AllReduce",
                        mybir.AluOpType.add,
                        replica_groups=PAIRS,
                        ins=[gsrc.opt()],
                        outs=[gsum_tri.opt()],
                    )

            # wv/wo for the C phase and x.T for the out phase stream after
            # the AR trigger / triangle stores, off G's critical DMA path.
            wv_t = []
            for i in range(DT):
                wt = w_tile("wv", i, D)
                nc.gpsimd.dma_start(out=wt[:], in_=wvT[ts(i, P), :])
                wv_t.append(wt)
            woh_t = []
            for i in range(DT):
                wt = w_tile("wo", i, OWN)
                nc.scalar.dma_start(out=wt[:], in_=woh[ts(i, P), :])
                woh_t.append(wt)
            xts = []
            for i in range(DT):
                xv = xt_pool.tile([P, H], bf16, name=f"xt{i}", tag="xt")
                nc.gpsimd.dma_start(out=xv[:], in_=xt[ts(i, P), :])
                xts.append(xv)

            # --- AT[j,d] = (w_q @ w_k.T).T, full width: fills the AR window ---
            ats = [
                at_pool.tile([P, D], bf16, name=f"at{j}", tag="at") for j in range(DT)
            ]
            for jt in range(DT):
                for dc in range(KC):
                    psum = ps_pool.tile([P, FREE], f32, name="psa", tag="ps")
                    for i in range(DT):
                        nc.tensor.matmul(
                            psum[:],
                            wk_t[i][:, ts(jt, P)],
                            wq_t[i][:, ts(dc, FREE)],
                            start=(i == 0),
                            stop=(i == DT - 1),
                        )
                    nc.vector.tensor_copy(ats[jt][:, ts(dc, FREE)], psum[:])

            # --- C[k, own cols] = (w_v @ w_o)[k, own] (640 wide) ---
            cs = []
            for kt in range(DT):
                ct = c_pool.tile([P, OWN], bf16, name=f"c{kt}", tag="c")
                for off, w in ((0, FREE), (FREE, OWN - FREE)):
                    psum = ps_pool.tile([P, w], f32, name="psc", tag="ps")
                    for l in range(DT):
                        nc.tensor.matmul(
                            psum[:],
                            wv_t[l][:, ts(kt, P)],
                            woh_t[l][:, off : off + w],
                            start=(l == 0),
                            stop=(l == DT - 1),
                        )
                    nc.vector.tensor_copy(ct[:, off : off + w], psum[:])
                cs.append(ct)

        # Filler matmuls: a small AR overrun past the AT phase would
        # otherwise cross the HAM MID window and re-throttle the PE.
        fps = ps_pool.tile([P, FREE], f32, name="fps", tag="ps")
        for _ in range(FILLER):
            nc.tensor.matmul(fps[:], wu[:, :P], wu[:], start=True, stop=True)

        # Late-phase pools, created after the setup pools release their SBUF.
        gf_pool = top.enter_context(tc.tile_pool(name="gf", bufs=DT))
        tl_pool = top.enter_context(tc.tile_pool(name="tl", bufs=DT - 1))
        r_pool = top.enter_context(tc.tile_pool(name="r", bufs=DT))
        m_pool = top.enter_context(tc.tile_pool(name="m", bufs=2 * DT))
        ot_pool = top.enter_context(tc.tile_pool(name="ot", bufs=6))

        # Summed triangle rows; spread the AR-gated loads over two queues.
        gts = []
        for jt in range(DT):
            w = (DT - jt) * P
            gt = gf_pool.tile([P, w], bf16, name=f"gt{jt}", tag="gf")
            eng = (nc.sync, nc.scalar)[jt % 2]
            eng.dma_start(out=gt[:], in_=gsum_tri[:, TRI_OFF[jt] : TRI_OFF[jt] + w])
            gts.append(gt)

        # Lower lhsT tiles of G, as PE transposes of the summed upper tiles.
        tlow = {}

        def emit_transposes():
            for jt in range(DT - 1):
                n = DT - 1 - jt
                tl = tl_pool.tile([P, n * P], bf16, name=f"tl{jt}", tag="tl")
                b0 = 0
                while b0 < n:
                    nb = min(FREE // P, n - b0)
                    pst = ps_pool.tile([P, nb * P], bf16, name="pstl", tag="ps")
                    for i in range(nb):
                        nc.tensor.transpose(
                            pst[:, ts(i, P)],
                            gts[jt][:, (b0 + i + 1) * P : (b0 + i + 2) * P],
                            ident[:],
                        )
                    nc.vector.tensor_copy(tl[:, b0 * P : (b0 + nb) * P], pst[:])
                    b0 += nb
                tlow[jt] = tl

        def g_lhsT(kt, jt):
            if kt <= jt:
                return gts[kt][:, (jt - kt) * P : (jt - kt + 1) * P]
            return tlow[jt][:, (kt - jt - 1) * P : (kt - jt) * P]

        # R[j,e] = (G @ C)[j,e], rotated cols [0:OWN). Row DT-1 uses only
        # upper/diag tiles, so it runs while the transposes still settle.
        r_order = [DT - 1] + list(range(DT - 1))
        rs = [None] * DT
        for pos, jt in enumerate(r_order):
            rt = r_pool.tile([P, OWN], bf16, name=f"r{jt}", tag="r")
            for off, w in ((0, FREE), (FREE, OWN - FREE)):
                psum = ps_pool.tile([P, w], f32, name="psr", tag="ps")
                for kt in range(DT):
                    nc.tensor.matmul(
                        psum[:],
                        g_lhsT(kt, jt),
                        cs[kt][:, off : off + w],
                        start=(kt == 0),
                        stop=(kt == DT - 1),
                    )
                nc.vector.tensor_copy(rt[:, off : off + w], psum[:])
            rs[jt] = rt
            if pos == 0:
                emit_transposes()

        # M[d,e] = (w_q @ w_k.T @ R)[d,e], rotated cols [0:OWN); the pair
        # exchanges slice [128:512) below while out runs on the PE.
        ms = []
        for dt_ in range(DT):
            mt = m_pool.tile([P, OWN], bf16, name=f"m{dt_}", tag="m")
            for off, w in ((0, FREE), (FREE, OWN - FREE)):
                psum = ps_pool.tile([P, w], f32, name="psm", tag="ps")
                for jt in range(DT):
                    nc.tensor.matmul(
                        psum[:],
                        ats[jt][:, ts(dt_, P)],
                        rs[jt][:, off : off + w],
                        start=(jt == 0),
                        stop=(jt == DT - 1),
                    )
                nc.vector.tensor_copy(mt[:, off : off + w], psum[:])
            ms.append(mt)
            # Masked staging: own slot zeroed, so the pair ReduceScatter
            # delivers exactly the peer's sent slice on both cores.
            for s in range(2):
                km = ot_pool.tile([P, SENDW], bf16, name="km", tag="ot")
                nc.vector.tensor_scalar_mul(
                    km[:], mt[:, SEND0 : SEND0 + SENDW], mb[:, s : s + 1]
                )
                eng = nc.scalar if s == 0 else nc.sync
                eng.dma_start(out=mstage[s, dt_], in_=km[:])
        nc.gpsimd.collective_compute(
            "ReduceScatter",
            mybir.AluOpType.add,
            replica_groups=PAIRS,
            ins=[mstage.opt()],
            outs=[mpeer.opt()],
        )
        mp = []
        for dt_ in range(DT):
            t_ = m_pool.tile([P, SENDW], bf16, name=f"mp{dt_}", tag="m")
            nc.gpsimd.dma_start(out=t_[:], in_=mpeer[dt_])
            mp.append(t_)

        # out[t,e] = sum_d x[t,d] M[d,e], own-half rows. The own 640 cols
        # run first so the PE is busy while the M exchange is in flight.
        pieces = ((0, FREE, None), (FREE, OWN - FREE, None), (OWN, SENDW, mp))
        for off, w, src in pieces:
            for tt in range(TT):
                psum = ps_pool.tile([P, w], f32, name="pso", tag="ps")
                for dt_ in range(DT):
                    rhs = src[dt_][:] if src is not None else ms[dt_][:, off : off + w]
                    nc.tensor.matmul(
                        psum[:],
                        xts[dt_][:, ts(tt, P)],
                        rhs,
                        start=(dt_ == 0),
                        stop=(dt_ == DT - 1),
                    )
                o = ot_pool.tile([P, w], bf16, name="ot", tag="ot")
                if tt % 2 == 0:
                    nc.scalar.copy(o[:], psum[:])
                    nc.scalar.dma_start(out=out[ts(tt, P), off : off + w], in_=o[:])
                else:
                    nc.vector.tensor_copy(o[:], psum[:])
                    nc.sync.dma_start(out=out[ts(tt, P), off : off + w], in_=o[:])


def _build():
    _install_axon_ntff_shim()
    import concourse.mybir as mybir
    import concourse.tile as tile
    from concourse import bacc

    f32 = mybir.dt.float32
    bf16 = mybir.dt.bfloat16
    nc = bacc.Bacc("TRN2", target_bir_lowering=False, debug=False, num_devices=NCORES)
    xn = nc.dram_tensor("xn", [H, D], bf16, kind="ExternalInput").ap()
    xt = nc.dram_tensor("xt", [D, H], bf16, kind="ExternalInput").ap()
    wqT = nc.dram_tensor("wqT", [D, D], bf16, kind="ExternalInput").ap()
    wkT = nc.dram_tensor("wkT", [D, D], bf16, kind="ExternalInput").ap()
    wvT = nc.dram_tensor("wvT", [D, D], bf16, kind="ExternalInput").ap()
    woh = nc.dram_tensor("woh", [D, OWN], bf16, kind="ExternalInput").ap()
    mask = nc.dram_tensor("mask", [P, 2], f32, kind="ExternalInput").ap()
    out = nc.dram_tensor("out", [H, D], bf16, kind="ExternalOutput").ap()

    with tile.TileContext(nc) as tc:
        _trace_kernel(tc, xn, xt, wqT, wkT, wvT, woh, mask, out)
    nc.compile()
    return nc


def kernel(x, w_q, w_k, w_v, w_o):
    global LAST_RESULTS
    import ml_dtypes
    from concourse import bass_utils

    if "nc" not in _STATE:
        _STATE["nc"] = _build()
    nc = _STATE["nc"]

    bf16 = ml_dtypes.bfloat16
    x = np.ascontiguousarray(x, dtype=np.float32)
    wqT = np.asarray(w_q, dtype=np.float32).T.astype(bf16)
    wkT = np.asarray(w_k, dtype=np.float32).T.astype(bf16)
    wvT = np.asarray(w_v, dtype=np.float32).T.astype(bf16)
    wof = np.asarray(w_o, dtype=np.float32)

    in_maps = []
    for core in range(NCORES):
        b, half = core // 2, core % 2
        xh = x[b, half * H : (half + 1) * H]
        m = np.zeros((P, 2), dtype=np.float32)
        m[:, 1 - half] = 1.0  # zero own slot; pair position == half
        # "Rotated" col e == natural col (e + 512h) mod 1024: each core's C
        # window is a host-side roll+slice of wo's columns.
        woh = np.ascontiguousarray(
            np.roll(wof, -512 * half, axis=1)[:, :OWN]
        ).astype(bf16)
        in_maps.append(
            {
                "xn": xh.astype(bf16),
                "xt": xh.T.astype(bf16),
                "wqT": wqT,
                "wkT": wkT,
                "wvT": wvT,
                "woh": woh,
                "mask": m,
            }
        )

    LAST_RESULTS = bass_utils.run_bass_kernel_spmd(
        nc, in_maps, core_ids=list(range(NCORES))
    )
    out = np.empty((B, T, D), dtype=np.float32)
    for core in range(NCORES):
        b, half = core // 2, core % 2
        res = np.asarray(LAST_RESULTS.results[core]["out"], dtype=np.float32)
        rows = slice(half * H, (half + 1) * H)
        # un-rotate: natural col c lives at rotated col (c - 512h) mod 1024
        out[b, rows] = np.roll(res, 512 * half, axis=1)
    return out


# revision 24
# speedup vs baseline: 1.1898x; 1.1898x over previous
"""Trainium2 Bass kernel: unnormalized single-head attention block.

Computes, for x [4, 4096, 1024] and w_q/w_k/w_v/w_o [1024, 1024] (all fp32):
    q = x @ w_q ; k = x @ w_k ; v = x @ w_v
    scores = q @ k.T            (no softmax)
    out = (scores @ v) @ w_o

No softmax, so the chain collapses to
    out_b = x_b @ [ w_q @ w_k.T @ (x_b.T @ x_b) @ w_v @ w_o ]
i.e. a Gram matrix G_b = x_b.T @ x_b plus a short chain of 1024^3 matmuls
(~90 GFLOP total instead of ~412 GFLOP).

Sharding: 8 NeuronCores = (4 batches) x (2 sequence halves), core = 2b + h.
Pair groups [[0,1],[2,3],[4,5],[6,7]] exchange G and M.

Measured system constants this schedule is built around:
  - the CC engine has a one-time ~40us startup before its first collective
    executes (and activates at ~20us); a tiny dummy pair-ReduceScatter
    prepays it so the real collectives start the moment they're triggered;
  - a pair collective costs ~6us + ~21us/MB of peer payload;
  - the PE sustains 1 matmul column/cycle at 2.4 GHz but re-throttles to
    1.2 GHz after long idles, so the schedule keeps it continuously fed.

Schedule (PE order):
  1. ~16 dummy matmuls warm the HAM clock gate while the x DMAs stream.
  2. G upper triangle only (G is symmetric): per 128-row tile jt, cols
     >= 128*jt, packed into a 1.125 MB triangle -> pairwise AllReduce.
     wqT tile loads are interleaved between the per-row staging stores on
     the scalar queue so neither blocks the other.
  3. C = w_v @ w_o[:, own 640 cols]: at the 640/384 R/M column split each
     core only consumes its own 640-column window of C, so the host
     pre-rolls/slices wo and C is computed at 640 width.
  4. AT = w_k @ w_q.T (27us, needed in full for M) fills the rest of the
     AllReduce window, hiding it completely.
  5. Post-AR: load the summed triangle, rebuild the 28 lower lhsT tiles
     with PE transposes (row 7 of R needs none, so it is emitted first).
  6. R = G @ C and M = AT.T @ R for rotated cols [0:640) -- "rotated" col
     e is natural col (e + 512h) mod 1024, realized purely by the host's
     wo rolling, which keeps the SPMD program rank-free. Both pair
     members lack exactly the peer's rotated cols [128:512), so that
     slice is exchanged via a masked pair ReduceScatter (own slot zeroed)
     while out = x_own @ M[:, 0:640) runs on the PE (34us of cover for a
     ~22us exchange).
  7. out psum is stored bf16 (host casts back to fp32), stores alternate
     scalar/sync queues; the host un-rotates odd cores' output columns.

Device math is bf16 with fp32 PSUM accumulation (rel err ~5.9e-3 vs fp32
reference). The host ships bf16 tensors directly so no on-device layout
changes or casts are needed.
"""

import contextlib
import ctypes
import os
import sys
import types

import numpy as np

B = 4
T = 4096
D = 1024
H = T // 2          # rows per core
P = 128             # SBUF partitions
NCORES = 8
DT = D // P         # 8 tiles along any 1024 dim
TT = H // P         # 16 own-half t-tiles
FREE = 512          # matmul moving free dim / PSUM bank width (fp32)
KC = D // FREE      # 2 free-dim chunks of 512 along a 1024 dim
PAIRS = [[0, 1], [2, 3], [4, 5], [6, 7]]
WARMUP = 16    # dummy matmuls to warm the HAM clock gate during the first DMAs
FILLER = 8     # post-AT dummy matmuls so an AR overrun can't re-throttle the PE
OWN = 704      # rotated M/R columns computed locally (peer computes the rest)
SEND0 = OWN - 512   # sent M slice [128:512) -- rank-free by pair symmetry
SENDW = D - OWN     # 384

_STATE = {}
LAST_RESULTS = None


def _install_axon_ntff_shim():
    """bass_utils(trace=True) under axon imports antenv.axon_hooks, which the
    agent image lacks. Provide the documented ctypes equivalent so tracing
    works; degrades to hook=None when the .so has no profile symbols."""
    try:
        import antenv.axon_hooks  # noqa: F401
        return
    except ImportError:
        pass

    so_path = "/opt/axon/libaxon_pjrt.so"

    def _make_hook():
        try:
            lib = ctypes.CDLL(so_path)
        except OSError:
            return None
        if not hasattr(lib, "axon_start_nrt_profile"):
            return None
        lib.axon_start_nrt_profile.argtypes = [
            ctypes.POINTER(ctypes.c_int64),
            ctypes.c_size_t,
        ]
        lib.axon_start_nrt_profile.restype = ctypes.c_int64
        lib.axon_stop_nrt_profile.argtypes = [ctypes.c_char_p]
        lib.axon_stop_nrt_profile.restype = ctypes.c_int64

        @contextlib.contextmanager
        def _hook(output_dir, device_ids):
            import jax

            jax.devices()
            if device_ids:
                ids = (ctypes.c_int64 * len(device_ids))(*device_ids)
                rc = lib.axon_start_nrt_profile(ids, len(device_ids))
            else:
                rc = lib.axon_start_nrt_profile(None, 0)
            if rc != 0:
                raise RuntimeError(f"axon_start_nrt_profile rc={rc}")
            try:
                yield
            finally:
                n = lib.axon_stop_nrt_profile(str(output_dir).encode())
                print(f"profile: {n} file(s) written to {output_dir}", file=sys.stderr)

        return _hook

    mod = types.ModuleType("antenv.axon_hooks")
    mod.get_axon_ntff_profile_hook = _make_hook
    mod.set_axon_ntff_profile_hook = lambda h: None
    sys.modules["antenv.axon_hooks"] = mod


def _trace_kernel(tc, xn, xt, wqT, wkT, wvT, woh, mask, out):
    import concourse.mybir as mybir
    from concourse.bass import ts

    nc = tc.nc
    f32 = mybir.dt.float32
    bf16 = mybir.dt.bfloat16

    with contextlib.ExitStack() as top:
        ps_pool = top.enter_context(tc.tile_pool(name="ps", bufs=8, space="PSUM"))
        dram_pool = top.enter_context(tc.tile_pool(name="cdram", bufs=2, space="DRAM"))
        at_pool = top.enter_context(tc.tile_pool(name="at", bufs=DT))
        c_pool = top.enter_context(tc.tile_pool(name="c", bufs=DT))
        xt_pool = top.enter_context(tc.tile_pool(name="xt", bufs=DT))

        # G triangle: row jt contributes cols >= jt*128, packed into a
        # 1.125 MB strip exchanged with a pairwise AllReduce.
        TRI_OFF = [0] * DT
        for r in range(1, DT):
            TRI_OFF[r] = TRI_OFF[r - 1] + (DT - (r - 1)) * P
        TRI_W = TRI_OFF[-1] + P  # 4608
        gsrc = dram_pool.tile([P, TRI_W], bf16, name="gsrc", tag="gsrc")
        gsum_tri = dram_pool.tile([P, TRI_W], bf16, name="gsumt", tag="gsum")

        # CC-engine warmup dummy collective staging.
        fsrc = dram_pool.tile([2, P, 4], bf16, name="fsrc", tag="flag")
        fsum = dram_pool.tile([P, 4], bf16, name="fsum", tag="flag")

        from concourse import masks

        id_pool = top.enter_context(tc.tile_pool(name="idp", bufs=2))
        wu_pool = top.enter_context(tc.tile_pool(name="wu", bufs=1))
        wu = wu_pool.tile([P, FREE], bf16, name="wu", tag="wu")
        nc.vector.memset(wu[:], 0.0)
        nc.sync.dma_start(out=fsrc[0], in_=wu[:, 0:4])
        nc.sync.dma_start(out=fsrc[1], in_=wu[:, 0:4])
        # Dummy collective: prepays the CC engine's one-time ~40us startup
        # while the input DMAs stream, so the real G-AR starts instantly.
        nc.gpsimd.collective_compute(
            "ReduceScatter",
            mybir.AluOpType.add,
            replica_groups=PAIRS,
            ins=[fsrc.opt()],
            outs=[fsum.opt()],
        )

        ident = id_pool.tile([P, P], bf16, name="ident", tag="id")
        masks.make_identity(nc, ident[:])

        # Pair-position mask for the M-slice exchange (own slot zeroed).
        mb = id_pool.tile([P, 2], f32, name="mb", tag="mb")
        nc.sync.dma_start(out=mb[:], in_=mask)
        mstage = dram_pool.tile([2, DT, P, SENDW], bf16, name="mstage", tag="mst")
        mpeer = dram_pool.tile([DT, P, SENDW], bf16, name="mpeer", tag="mpr")

        wps = ps_pool.tile([P, FREE], f32, name="wps", tag="ps")
        for _ in range(WARMUP):
            nc.tensor.matmul(wps[:], wu[:, :P], wu[:], start=True, stop=True)

        with contextlib.ExitStack() as setup:
            xn_pool = setup.enter_context(tc.tile_pool(name="xn", bufs=TT))
            w_pool = setup.enter_context(tc.tile_pool(name="w", bufs=4 * DT))

            # x tiles: the only DMAs G needs, split over sync/scalar.
            xns = []
            for t in range(TT):
                xv = xn_pool.tile([P, D], bf16, name=f"xn{t}", tag="xn")
                eng = nc.sync if t % 2 == 0 else nc.scalar
                eng.dma_start(out=xv[:], in_=xn[ts(t, P), :])
                xns.append(xv)

            def w_tile(tag, i, width):
                return w_pool.tile([P, width], bf16, name=f"{tag}{i}", tag="w")

            # Weights stream behind the x tiles: wv/wk on sync, woh on
            # scalar; wqT tile loads are interleaved with the G staging
            # stores below. (gpsimd stays clear: DMAs behind a collective
            # trigger on it have been observed to stall until it executes.)
            wv_t = []
            for i in range(DT):
                wt = w_tile("wv", i, D)
                nc.sync.dma_start(out=wt[:], in_=wvT[ts(i, P), :])
                wv_t.append(wt)
            woh_t = []
            for i in range(DT):
                wt = w_tile("wo", i, OWN)
                nc.scalar.dma_start(out=wt[:], in_=woh[ts(i, P), :])
                woh_t.append(wt)
            wk_t = []
            for i in range(DT):
                wt = w_tile("wk", i, D)
                nc.sync.dma_start(out=wt[:], in_=wkT[ts(i, P), :])
                wk_t.append(wt)
            wq_t = [w_tile("wq", i, D) for i in range(DT)]

            # --- own-half Gram matrix G[j,k] = sum_t x[t,j] x[t,k] ---
            # Upper triangle only; lower tiles are rebuilt after the AR.
            # wqT tile loads are interleaved between the per-row staging
            # stores so the scalar queue serves both without blocking.
            gown_pool = setup.enter_context(tc.tile_pool(name="gown", bufs=DT))
            gown = [
                gown_pool.tile([P, D], bf16, name=f"go{j}", tag="gown")
                for j in range(DT)
            ]
            # t-outer accumulation in two waves of <=8 open PSUM banks: each
            # matmul needs only x tile t, so G is paced by tile arrival (a
            # chunk-outer loop would stall every 3.4us chain on all 16 DMAs).
            waves = [range(0, DT // 2), range(DT // 2, DT)]
            for wjts in waves:
                chunks = []
                for jt in wjts:
                    off = jt * P
                    while off < D:
                        w = min(FREE, D - off)
                        chunks.append((jt, off, w))
                        off += w
                psums = [
                    ps_pool.tile([P, w], f32, name="psg", tag="ps")
                    for (_, _, w) in chunks
                ]
                for t in range(TT):
                    for idx, (jt, off, w) in enumerate(chunks):
                        nc.tensor.matmul(
                            psums[idx][:],
                            xns[t][:, ts(jt, P)],
                            xns[t][:, off : off + w],
                            start=(t == 0),
                            stop=(t == TT - 1),
                        )
                for idx, (jt, off, w) in enumerate(chunks):
                    nc.vector.tensor_copy(gown[jt][:, off : off + w], psums[idx][:])
                for jt in wjts:
                    rw = (DT - jt) * P
                    sl = slice(TRI_OFF[jt], TRI_OFF[jt] + rw)
                    nc.scalar.dma_start(out=gsrc[:, sl], in_=gown[jt][:, jt * P :])
                    nc.scalar.dma_start(out=wq_t[jt][:], in_=wqT[ts(jt, P), :])
            nc.gpsimd.collective_compute(
                "AllReduce",
                mybir.AluOpType.add,
                replica_groups=PAIRS,
                ins=[gsrc.opt()],
                outs=[gsum_tri.opt()],
            )

            # x.T tiles for the final out = x @ M matmul ride gpsimd after
            # the AR trigger; they are needed only in the out phase.
            xts = []
            for i in range(DT):
                xv = xt_pool.tile([P, H], bf16, name=f"xt{i}", tag="xt")
                nc.gpsimd.dma_start(out=xv[:], in_=xt[ts(i, P), :])
                xts.append(xv)

            # --- C[k, own cols] = (w_v @ w_o)[k, own] (OWN wide) ---
            cs = []
            for kt in range(DT):
                ct = c_pool.tile([P, OWN], bf16, name=f"c{kt}", tag="c")
                for off, w in ((0, FREE), (FREE, OWN - FREE)):
                    psum = ps_pool.tile([P, w], f32, name="psc", tag="ps")
                    for l in range(DT):
                        nc.tensor.matmul(
                            psum[:],
                            wv_t[l][:, ts(kt, P)],
                            woh_t[l][:, off : off + w],
                            start=(l == 0),
                            stop=(l == DT - 1),
                        )
                    nc.vector.tensor_copy(ct[:, off : off + w], psum[:])
                cs.append(ct)

            # --- AT[j,d] = (w_q @ w_k.T).T, full width: fills the AR window ---
            ats = [
                at_pool.tile([P, D], bf16, name=f"at{j}", tag="at") for j in range(DT)
            ]
            for jt in range(DT):
                for dc in range(KC):
                    psum = ps_pool.tile([P, FREE], f32, name="psa", tag="ps")
                    for i in range(DT):
                        nc.tensor.matmul(
                            psum[:],
                            wk_t[i][:, ts(jt, P)],
                            wq_t[i][:, ts(dc, FREE)],
                            start=(i == 0),
                            stop=(i == DT - 1),
                        )
                    nc.vector.tensor_copy(ats[jt][:, ts(dc, FREE)], psum[:])

        # Filler matmuls: a small AR overrun past the AT phase would
        # otherwise cross the HAM MID window and re-throttle the PE.
        fps = ps_pool.tile([P, FREE], f32, name="fps", tag="ps")
        for _ in range(FILLER):
            nc.tensor.matmul(fps[:], wu[:, :P], wu[:], start=True, stop=True)

        # Late-phase pools, created after the setup pools release their SBUF.
        gf_pool = top.enter_context(tc.tile_pool(name="gf", bufs=DT))
        tl_pool = top.enter_context(tc.tile_pool(name="tl", bufs=DT - 1))
        r_pool = top.enter_context(tc.tile_pool(name="r", bufs=DT))
        m_pool = top.enter_context(tc.tile_pool(name="m", bufs=2 * DT))
        ot_pool = top.enter_context(tc.tile_pool(name="ot", bufs=6))

        # Summed triangle rows; spread the AR-gated loads over two queues.
        gts = []
        for jt in range(DT):
            w = (DT - jt) * P
            gt = gf_pool.tile([P, w], bf16, name=f"gt{jt}", tag="gf")
            eng = (nc.sync, nc.scalar)[jt % 2]
            eng.dma_start(out=gt[:], in_=gsum_tri[:, TRI_OFF[jt] : TRI_OFF[jt] + w])
            gts.append(gt)

        # Lower lhsT tiles of G, as PE transposes of the summed upper tiles.
        tlow = {}

        def emit_transposes():
            for jt in range(DT - 1):
                n = DT - 1 - jt
                tl = tl_pool.tile([P, n * P], bf16, name=f"tl{jt}", tag="tl")
                b0 = 0
                while b0 < n:
                    nb = min(FREE // P, n - b0)
                    pst = ps_pool.tile([P, nb * P], bf16, name="pstl", tag="ps")
                    for i in range(nb):
                        nc.tensor.transpose(
                            pst[:, ts(i, P)],
                            gts[jt][:, (b0 + i + 1) * P : (b0 + i + 2) * P],
                            ident[:],
                        )
                    nc.vector.tensor_copy(tl[:, b0 * P : (b0 + nb) * P], pst[:])
                    b0 += nb
                tlow[jt] = tl

        def g_lhsT(kt, jt):
            if kt <= jt:
                return gts[kt][:, (jt - kt) * P : (jt - kt + 1) * P]
            return tlow[jt][:, (kt - jt - 1) * P : (kt - jt) * P]

        # R[j,e] = (G @ C)[j,e], rotated cols [0:OWN). Row DT-1 uses only
        # upper/diag tiles, so it runs while the transposes still settle.
        r_order = [DT - 1] + list(range(DT - 1))
        rs = [None] * DT
        for pos, jt in enumerate(r_order):
            rt = r_pool.tile([P, OWN], bf16, name=f"r{jt}", tag="r")
            for off, w in ((0, FREE), (FREE, OWN - FREE)):
                psum = ps_pool.tile([P, w], f32, name="psr", tag="ps")
                for kt in range(DT):
                    nc.tensor.matmul(
                        psum[:],
                        g_lhsT(kt, jt),
                        cs[kt][:, off : off + w],
                        start=(kt == 0),
                        stop=(kt == DT - 1),
                    )
                nc.vector.tensor_copy(rt[:, off : off + w], psum[:])
            rs[jt] = rt
            if pos == 0:
                emit_transposes()

        # M[d,e] = (w_q @ w_k.T @ R)[d,e], rotated cols [0:OWN); the pair
        # exchanges slice [128:512) below while out runs on the PE.
        ms = []
        for dt_ in range(DT):
            mt = m_pool.tile([P, OWN], bf16, name=f"m{dt_}", tag="m")
            for off, w in ((0, FREE), (FREE, OWN - FREE)):
                psum = ps_pool.tile([P, w], f32, name="psm", tag="ps")
                for jt in range(DT):
                    nc.tensor.matmul(
                        psum[:],
                        ats[jt][:, ts(dt_, P)],
                        rs[jt][:, off : off + w],
                        start=(jt == 0),
                        stop=(jt == DT - 1),
                    )
                nc.vector.tensor_copy(mt[:, off : off + w], psum[:])
            ms.append(mt)
            # Masked staging: own slot zeroed, so the pair ReduceScatter
            # delivers exactly the peer's sent slice on both cores.
            for s in range(2):
                km = ot_pool.tile([P, SENDW], bf16, name="km", tag="ot")
                nc.vector.tensor_scalar_mul(
                    km[:], mt[:, SEND0 : SEND0 + SENDW], mb[:, s : s + 1]
                )
                eng = nc.scalar if s == 0 else nc.sync
                eng.dma_start(out=mstage[s, dt_], in_=km[:])
        nc.gpsimd.collective_compute(
            "ReduceScatter",
            mybir.AluOpType.add,
            replica_groups=PAIRS,
            ins=[mstage.opt()],
            outs=[mpeer.opt()],
        )
        mp = []
        for dt_ in range(DT):
            t_ = m_pool.tile([P, SENDW], bf16, name=f"mp{dt_}", tag="m")
            nc.gpsimd.dma_start(out=t_[:], in_=mpeer[dt_])
            mp.append(t_)

        # out[t,e] = sum_d x[t,d] M[d,e], own-half rows. The own 640 cols
        # run first so the PE is busy while the M exchange is in flight.
        pieces = ((0, FREE, None), (FREE, OWN - FREE, None), (OWN, SENDW, mp))
        for off, w, src in pieces:
            for tt in range(TT):
                psum = ps_pool.tile([P, w], f32, name="pso", tag="ps")
                for dt_ in range(DT):
                    rhs = src[dt_][:] if src is not None else ms[dt_][:, off : off + w]
                    nc.tensor.matmul(
                        psum[:],
                        xts[dt_][:, ts(tt, P)],
                        rhs,
                        start=(dt_ == 0),
                        stop=(dt_ == DT - 1),
                    )
                o = ot_pool.tile([P, w], bf16, name="ot", tag="ot")
                if tt % 2 == 0:
                    nc.scalar.copy(o[:], psum[:])
                    nc.scalar.dma_start(out=out[ts(tt, P), off : off + w], in_=o[:])
                else:
                    nc.vector.tensor_copy(o[:], psum[:])
                    nc.sync.dma_start(out=out[ts(tt, P), off : off + w], in_=o[:])


def _build():
    _install_axon_ntff_shim()
    import concourse.mybir as mybir
    import concourse.tile as tile
    from concourse import bacc

    f32 = mybir.dt.float32
    bf16 = mybir.dt.bfloat16
    nc = bacc.Bacc("TRN2", target_bir_lowering=False, debug=False, num_devices=NCORES)
    xn = nc.dram_tensor("xn", [H, D], bf16, kind="ExternalInput").ap()
    xt = nc.dram_tensor("xt", [D, H], bf16, kind="ExternalInput").ap()
    wqT = nc.dram_tensor("wqT", [D, D], bf16, kind="ExternalInput").ap()
    wkT = nc.dram_tensor("wkT", [D, D], bf16, kind="ExternalInput").ap()
    wvT = nc.dram_tensor("wvT", [D, D], bf16, kind="ExternalInput").ap()
    woh = nc.dram_tensor("woh", [D, OWN], bf16, kind="ExternalInput").ap()
    mask = nc.dram_tensor("mask", [P, 2], f32, kind="ExternalInput").ap()
    out = nc.dram_tensor("out", [H, D], bf16, kind="ExternalOutput").ap()

    with tile.TileContext(nc) as tc:
        _trace_kernel(tc, xn, xt, wqT, wkT, wvT, woh, mask, out)
    nc.compile()
    return nc


def kernel(x, w_q, w_k, w_v, w_o):
    global LAST_RESULTS
    import ml_dtypes
    from concourse import bass_utils

    if "nc" not in _STATE:
        _STATE["nc"] = _build()
    nc = _STATE["nc"]

    bf16 = ml_dtypes.bfloat16
    x = np.ascontiguousarray(x, dtype=np.float32)
    wqT = np.asarray(w_q, dtype=np.float32).T.astype(bf16)
    wkT = np.asarray(w_k, dtype=np.float32).T.astype(bf16)
    wvT = np.asarray(w_v, dtype=np.float32).T.astype(bf16)
    wof = np.asarray(w_o, dtype=np.float32)

    in_maps = []
    for core in range(NCORES):
        b, half = core // 2, core % 2
        xh = x[b, half * H : (half + 1) * H]
        m = np.zeros((P, 2), dtype=np.float32)
        m[:, 1 - half] = 1.0  # zero own slot; pair position == half
        # "Rotated" col e == natural col (e + 512h) mod 1024: each core's C
        # window is a host-side roll+slice of wo's columns.
        woh = np.ascontiguousarray(
            np.roll(wof, -512 * half, axis=1)[:, :OWN]
        ).astype(bf16)
        in_maps.append(
            {
                "xn": xh.astype(bf16),
                "xt": xh.T.astype(bf16),
                "wqT": wqT,
                "wkT": wkT,
                "wvT": wvT,
                "woh": woh,
                "mask": m,
            }
        )

    LAST_RESULTS = bass_utils.run_bass_kernel_spmd(
        nc, in_maps, core_ids=list(range(NCORES))
    )
    out = np.empty((B, T, D), dtype=np.float32)
    for core in range(NCORES):
        b, half = core // 2, core % 2
        res = np.asarray(LAST_RESULTS.results[core]["out"], dtype=np.float32)
        rows = slice(half * H, (half + 1) * H)
        # un-rotate: natural col c lives at rotated col (c - 512h) mod 1024
        out[b, rows] = np.roll(res, 512 * half, axis=1)
    return out


# revision 25
# speedup vs baseline: 1.1998x; 1.0084x over previous
"""Trainium2 Bass kernel: unnormalized single-head attention block.

Computes, for x [4, 4096, 1024] and w_q/w_k/w_v/w_o [1024, 1024] (all fp32):
    q = x @ w_q ; k = x @ w_k ; v = x @ w_v
    scores = q @ k.T            (no softmax)
    out = (scores @ v) @ w_o

No softmax, so the chain collapses to
    out_b = x_b @ [ w_q @ w_k.T @ (x_b.T @ x_b) @ w_v @ w_o ]
i.e. a Gram matrix G_b = x_b.T @ x_b plus a short chain of 1024^3 matmuls
(~90 GFLOP total instead of ~412 GFLOP).

Sharding: 8 NeuronCores = (4 batches) x (2 sequence halves), core = 2b + h.
Pair groups [[0,1],[2,3],[4,5],[6,7]] exchange G and M.

Measured system constants this schedule is built around:
  - the CC engine has a one-time ~40us startup before its first collective
    executes (and activates at ~20us); a tiny dummy pair-ReduceScatter
    prepays it so the real collectives start the moment they're triggered;
  - a pair collective costs ~6us + ~21us/MB of peer payload;
  - the PE sustains 1 matmul column/cycle at 2.4 GHz but re-throttles to
    1.2 GHz after long idles, so the schedule keeps it continuously fed.

Schedule (PE order):
  1. ~16 dummy matmuls warm the HAM clock gate while the x DMAs stream.
  2. G upper triangle only (G is symmetric): per 128-row tile jt, cols
     >= 128*jt, packed into a 1.125 MB triangle -> pairwise AllReduce.
     wqT tile loads are interleaved between the per-row staging stores on
     the scalar queue so neither blocks the other.
  3. C = w_v @ w_o[:, own 640 cols]: at the 640/384 R/M column split each
     core only consumes its own 640-column window of C, so the host
     pre-rolls/slices wo and C is computed at 640 width.
  4. AT = w_k @ w_q.T (27us, needed in full for M) fills the rest of the
     AllReduce window, hiding it completely.
  5. Post-AR: load the summed triangle, rebuild the 28 lower lhsT tiles
     with PE transposes (row 7 of R needs none, so it is emitted first).
  6. R = G @ C and M = AT.T @ R for rotated cols [0:640) -- "rotated" col
     e is natural col (e + 512h) mod 1024, realized purely by the host's
     wo rolling, which keeps the SPMD program rank-free. Both pair
     members lack exactly the peer's rotated cols [128:512), so that
     slice is exchanged via a masked pair ReduceScatter (own slot zeroed)
     while out = x_own @ M[:, 0:640) runs on the PE (34us of cover for a
     ~22us exchange).
  7. out psum is stored bf16 (host casts back to fp32), stores alternate
     scalar/sync queues; the host un-rotates odd cores' output columns.

Device math is bf16 with fp32 PSUM accumulation (rel err ~5.9e-3 vs fp32
reference). The host ships bf16 tensors directly so no on-device layout
changes or casts are needed.
"""

import contextlib
import ctypes
import os
import sys
import types

import numpy as np

B = 4
T = 4096
D = 1024
H = T // 2          # rows per core
P = 128             # SBUF partitions
NCORES = 8
DT = D // P         # 8 tiles along any 1024 dim
TT = H // P         # 16 own-half t-tiles
FREE = 512          # matmul moving free dim / PSUM bank width (fp32)
KC = D // FREE      # 2 free-dim chunks of 512 along a 1024 dim
PAIRS = [[0, 1], [2, 3], [4, 5], [6, 7]]
WARMUP = 16    # dummy matmuls to warm the HAM clock gate during the first DMAs
FILLER = 8     # post-AT dummy matmuls so an AR overrun can't re-throttle the PE
OWN = 704      # rotated M/R columns computed locally (peer computes the rest)
SEND0 = OWN - 512   # sent M slice [128:512) -- rank-free by pair symmetry
SENDW = D - OWN     # 384

_STATE = {}
LAST_RESULTS = None


def _install_axon_ntff_shim():
    """bass_utils(trace=True) under axon imports antenv.axon_hooks, which the
    agent image lacks. Provide the documented ctypes equivalent so tracing
    works; degrades to hook=None when the .so has no profile symbols."""
    try:
        import antenv.axon_hooks  # noqa: F401
        return
    except ImportError:
        pass

    so_path = "/opt/axon/libaxon_pjrt.so"

    def _make_hook():
        try:
            lib = ctypes.CDLL(so_path)
        except OSError:
            return None
        if not hasattr(lib, "axon_start_nrt_profile"):
            return None
        lib.axon_start_nrt_profile.argtypes = [
            ctypes.POINTER(ctypes.c_int64),
            ctypes.c_size_t,
        ]
        lib.axon_start_nrt_profile.restype = ctypes.c_int64
        lib.axon_stop_nrt_profile.argtypes = [ctypes.c_char_p]
        lib.axon_stop_nrt_profile.restype = ctypes.c_int64

        @contextlib.contextmanager
        def _hook(output_dir, device_ids):
            import jax

            jax.devices()
            if device_ids:
                ids = (ctypes.c_int64 * len(device_ids))(*device_ids)
                rc = lib.axon_start_nrt_profile(ids, len(device_ids))
            else:
                rc = lib.axon_start_nrt_profile(None, 0)
            if rc != 0:
                raise RuntimeError(f"axon_start_nrt_profile rc={rc}")
            try:
                yield
            finally:
                n = lib.axon_stop_nrt_profile(str(output_dir).encode())
                print(f"profile: {n} file(s) written to {output_dir}", file=sys.stderr)

        return _hook

    mod = types.ModuleType("antenv.axon_hooks")
    mod.get_axon_ntff_profile_hook = _make_hook
    mod.set_axon_ntff_profile_hook = lambda h: None
    sys.modules["antenv.axon_hooks"] = mod


def _trace_kernel(tc, xn, xt, wqT, wkT, wvT, woh, mask, out):
    import concourse.mybir as mybir
    from concourse.bass import ts

    nc = tc.nc
    f32 = mybir.dt.float32
    bf16 = mybir.dt.bfloat16

    with contextlib.ExitStack() as top:
        ps_pool = top.enter_context(tc.tile_pool(name="ps", bufs=8, space="PSUM"))
        dram_pool = top.enter_context(tc.tile_pool(name="cdram", bufs=2, space="DRAM"))
        at_pool = top.enter_context(tc.tile_pool(name="at", bufs=DT))
        c_pool = top.enter_context(tc.tile_pool(name="c", bufs=DT))
        xt_pool = top.enter_context(tc.tile_pool(name="xt", bufs=DT))

        # G triangle: row jt contributes cols >= jt*128, packed into a
        # 1.125 MB strip exchanged with a pairwise AllReduce.
        TRI_OFF = [0] * DT
        for r in range(1, DT):
            TRI_OFF[r] = TRI_OFF[r - 1] + (DT - (r - 1)) * P
        TRI_W = TRI_OFF[-1] + P  # 4608
        gsrc = dram_pool.tile([P, TRI_W], bf16, name="gsrc", tag="gsrc")
        gsum_tri = dram_pool.tile([P, TRI_W], bf16, name="gsumt", tag="gsum")

        # CC-engine warmup dummy collective staging.
        fsrc = dram_pool.tile([2, P, 4], bf16, name="fsrc", tag="flag")
        fsum = dram_pool.tile([P, 4], bf16, name="fsum", tag="flag")

        from concourse import masks

        id_pool = top.enter_context(tc.tile_pool(name="idp", bufs=2))
        wu_pool = top.enter_context(tc.tile_pool(name="wu", bufs=1))
        wu = wu_pool.tile([P, FREE], bf16, name="wu", tag="wu")
        nc.vector.memset(wu[:], 0.0)
        nc.sync.dma_start(out=fsrc[0], in_=wu[:, 0:4])
        nc.sync.dma_start(out=fsrc[1], in_=wu[:, 0:4])
        # Dummy collective: prepays the CC engine's one-time ~40us startup
        # while the input DMAs stream, so the real G-AR starts instantly.
        nc.gpsimd.collective_compute(
            "ReduceScatter",
            mybir.AluOpType.add,
            replica_groups=PAIRS,
            ins=[fsrc.opt()],
            outs=[fsum.opt()],
        )

        ident = id_pool.tile([P, P], bf16, name="ident", tag="id")
        masks.make_identity(nc, ident[:])

        # Pair-position mask for the M-slice exchange (own slot zeroed).
        mb = id_pool.tile([P, 2], f32, name="mb", tag="mb")
        nc.sync.dma_start(out=mb[:], in_=mask)
        mstage = dram_pool.tile([2, DT, P, SENDW], bf16, name="mstage", tag="mst")
        mpeer = dram_pool.tile([DT, P, SENDW], bf16, name="mpeer", tag="mpr")

        wps = ps_pool.tile([P, FREE], f32, name="wps", tag="ps")
        for _ in range(WARMUP):
            nc.tensor.matmul(wps[:], wu[:, :P], wu[:], start=True, stop=True)

        with contextlib.ExitStack() as setup:
            xn_pool = setup.enter_context(tc.tile_pool(name="xn", bufs=TT))
            w_pool = setup.enter_context(tc.tile_pool(name="w", bufs=4 * DT))

            # x tiles: the only DMAs G needs, split over sync/scalar.
            xns = []
            for t in range(TT):
                xv = xn_pool.tile([P, D], bf16, name=f"xn{t}", tag="xn")
                eng = nc.sync if t % 2 == 0 else nc.scalar
                eng.dma_start(out=xv[:], in_=xn[ts(t, P), :])
                xns.append(xv)

            def w_tile(tag, i, width):
                return w_pool.tile([P, width], bf16, name=f"{tag}{i}", tag="w")

            # Weights stream behind the x tiles: wv/wk on sync, woh on
            # scalar; wqT tile loads are interleaved with the G staging
            # stores below. (gpsimd stays clear: DMAs behind a collective
            # trigger on it have been observed to stall until it executes.)
            wv_t = []
            for i in range(DT):
                wt = w_tile("wv", i, D)
                nc.sync.dma_start(out=wt[:], in_=wvT[ts(i, P), :])
                wv_t.append(wt)
            woh_t = []
            for i in range(DT):
                wt = w_tile("wo", i, OWN)
                nc.scalar.dma_start(out=wt[:], in_=woh[ts(i, P), :])
                woh_t.append(wt)
            wk_t = []
            for i in range(DT):
                wt = w_tile("wk", i, D)
                nc.sync.dma_start(out=wt[:], in_=wkT[ts(i, P), :])
                wk_t.append(wt)
            wq_t = [w_tile("wq", i, D) for i in range(DT)]

            # --- own-half Gram matrix G[j,k] = sum_t x[t,j] x[t,k] ---
            # Upper triangle only; lower tiles are rebuilt after the AR.
            # wqT tile loads are interleaved between the per-row staging
            # stores so the scalar queue serves both without blocking.
            gown_pool = setup.enter_context(tc.tile_pool(name="gown", bufs=DT))
            gown = [
                gown_pool.tile([P, D], bf16, name=f"go{j}", tag="gown")
                for j in range(DT)
            ]
            # t-outer accumulation in two waves of <=8 open PSUM banks: each
            # matmul needs only x tile t, so G is paced by tile arrival (a
            # chunk-outer loop would stall every 3.4us chain on all 16 DMAs).
            waves = [range(0, DT // 2), range(DT // 2, DT)]
            for wjts in waves:
                chunks = []
                for jt in wjts:
                    off = jt * P
                    while off < D:
                        w = min(FREE, D - off)
                        chunks.append((jt, off, w))
                        off += w
                psums = [
                    ps_pool.tile([P, w], f32, name="psg", tag="ps")
                    for (_, _, w) in chunks
                ]
                for t in range(TT):
                    for idx, (jt, off, w) in enumerate(chunks):
                        nc.tensor.matmul(
                            psums[idx][:],
                            xns[t][:, ts(jt, P)],
                            xns[t][:, off : off + w],
                            start=(t == 0),
                            stop=(t == TT - 1),
                        )
                for idx, (jt, off, w) in enumerate(chunks):
                    nc.vector.tensor_copy(gown[jt][:, off : off + w], psums[idx][:])
                for jt in wjts:
                    rw = (DT - jt) * P
                    sl = slice(TRI_OFF[jt], TRI_OFF[jt] + rw)
                    # Staging rides the otherwise-empty gpsimd queue so the
                    # AR triggers right at G-end instead of ~25us later
                    # behind the scalar queue's weight traffic.
                    nc.gpsimd.dma_start(out=gsrc[:, sl], in_=gown[jt][:, jt * P :])
                    nc.scalar.dma_start(out=wq_t[jt][:], in_=wqT[ts(jt, P), :])
            nc.gpsimd.collective_compute(
                "AllReduce",
                mybir.AluOpType.add,
                replica_groups=PAIRS,
                ins=[gsrc.opt()],
                outs=[gsum_tri.opt()],
            )

            # x.T tiles for the final out = x @ M matmul ride gpsimd after
            # the AR trigger; they are needed only in the out phase.
            xts = []
            for i in range(DT):
                xv = xt_pool.tile([P, H], bf16, name=f"xt{i}", tag="xt")
                nc.gpsimd.dma_start(out=xv[:], in_=xt[ts(i, P), :])
                xts.append(xv)

            # --- C[k, own cols] = (w_v @ w_o)[k, own] (OWN wide) ---
            cs = []
            for kt in range(DT):
                ct = c_pool.tile([P, OWN], bf16, name=f"c{kt}", tag="c")
                for off, w in ((0, FREE), (FREE, OWN - FREE)):
                    psum = ps_pool.tile([P, w], f32, name="psc", tag="ps")
                    for l in range(DT):
                        nc.tensor.matmul(
                            psum[:],
                            wv_t[l][:, ts(kt, P)],
                            woh_t[l][:, off : off + w],
                            start=(l == 0),
                            stop=(l == DT - 1),
                        )
                    nc.vector.tensor_copy(ct[:, off : off + w], psum[:])
                cs.append(ct)

            # --- AT[j,d] = (w_q @ w_k.T).T, full width: fills the AR window ---
            ats = [
                at_pool.tile([P, D], bf16, name=f"at{j}", tag="at") for j in range(DT)
            ]
            for jt in range(DT):
                for dc in range(KC):
                    psum = ps_pool.tile([P, FREE], f32, name="psa", tag="ps")
                    for i in range(DT):
                        nc.tensor.matmul(
                            psum[:],
                            wk_t[i][:, ts(jt, P)],
                            wq_t[i][:, ts(dc, FREE)],
                            start=(i == 0),
                            stop=(i == DT - 1),
                        )
                    nc.vector.tensor_copy(ats[jt][:, ts(dc, FREE)], psum[:])

        # Filler matmuls: a small AR overrun past the AT phase would
        # otherwise cross the HAM MID window and re-throttle the PE.
        fps = ps_pool.tile([P, FREE], f32, name="fps", tag="ps")
        for _ in range(FILLER):
            nc.tensor.matmul(fps[:], wu[:, :P], wu[:], start=True, stop=True)

        # Late-phase pools, created after the setup pools release their SBUF.
        gf_pool = top.enter_context(tc.tile_pool(name="gf", bufs=DT))
        tl_pool = top.enter_context(tc.tile_pool(name="tl", bufs=DT - 1))
        r_pool = top.enter_context(tc.tile_pool(name="r", bufs=DT))
        m_pool = top.enter_context(tc.tile_pool(name="m", bufs=2 * DT))
        ot_pool = top.enter_context(tc.tile_pool(name="ot", bufs=6))

        # Summed triangle rows; spread the AR-gated loads over two queues.
        gts = []
        for jt in range(DT):
            w = (DT - jt) * P
            gt = gf_pool.tile([P, w], bf16, name=f"gt{jt}", tag="gf")
            eng = (nc.sync, nc.scalar)[jt % 2]
            eng.dma_start(out=gt[:], in_=gsum_tri[:, TRI_OFF[jt] : TRI_OFF[jt] + w])
            gts.append(gt)

        # Lower lhsT tiles of G, as PE transposes of the summed upper tiles.
        tlow = {}

        def emit_transposes():
            for jt in range(DT - 1):
                n = DT - 1 - jt
                tl = tl_pool.tile([P, n * P], bf16, name=f"tl{jt}", tag="tl")
                b0 = 0
                while b0 < n:
                    nb = min(FREE // P, n - b0)
                    pst = ps_pool.tile([P, nb * P], bf16, name="pstl", tag="ps")
                    for i in range(nb):
                        nc.tensor.transpose(
                            pst[:, ts(i, P)],
                            gts[jt][:, (b0 + i + 1) * P : (b0 + i + 2) * P],
                            ident[:],
                        )
                    nc.vector.tensor_copy(tl[:, b0 * P : (b0 + nb) * P], pst[:])
                    b0 += nb
                tlow[jt] = tl

        def g_lhsT(kt, jt):
            if kt <= jt:
                return gts[kt][:, (jt - kt) * P : (jt - kt + 1) * P]
            return tlow[jt][:, (kt - jt - 1) * P : (kt - jt) * P]

        # R[j,e] = (G @ C)[j,e], rotated cols [0:OWN). Row DT-1 uses only
        # upper/diag tiles, so it runs while the transposes still settle.
        r_order = [DT - 1] + list(range(DT - 1))
        rs = [None] * DT
        for pos, jt in enumerate(r_order):
            rt = r_pool.tile([P, OWN], bf16, name=f"r{jt}", tag="r")
            for off, w in ((0, FREE), (FREE, OWN - FREE)):
                psum = ps_pool.tile([P, w], f32, name="psr", tag="ps")
                for kt in range(DT):
                    nc.tensor.matmul(
                        psum[:],
                        g_lhsT(kt, jt),
                        cs[kt][:, off : off + w],
                        start=(kt == 0),
                        stop=(kt == DT - 1),
                    )
                nc.vector.tensor_copy(rt[:, off : off + w], psum[:])
            rs[jt] = rt
            if pos == 0:
                emit_transposes()

        # M[d,e] = (w_q @ w_k.T @ R)[d,e], rotated cols [0:OWN); the pair
        # exchanges slice [128:512) below while out runs on the PE.
        ms = []
        for dt_ in range(DT):
            mt = m_pool.tile([P, OWN], bf16, name=f"m{dt_}", tag="m")
            for off, w in ((0, FREE), (FREE, OWN - FREE)):
                psum = ps_pool.tile([P, w], f32, name="psm", tag="ps")
                for jt in range(DT):
                    nc.tensor.matmul(
                        psum[:],
                        ats[jt][:, ts(dt_, P)],
                        rs[jt][:, off : off + w],
                        start=(jt == 0),
                        stop=(jt == DT - 1),
                    )
                nc.vector.tensor_copy(mt[:, off : off + w], psum[:])
            ms.append(mt)
            # Masked staging: own slot zeroed, so the pair ReduceScatter
            # delivers exactly the peer's sent slice on both cores.
            for s in range(2):
                km = ot_pool.tile([P, SENDW], bf16, name="km", tag="ot")
                nc.vector.tensor_scalar_mul(
                    km[:], mt[:, SEND0 : SEND0 + SENDW], mb[:, s : s + 1]
                )
                eng = nc.scalar if s == 0 else nc.sync
                eng.dma_start(out=mstage[s, dt_], in_=km[:])
        nc.gpsimd.collective_compute(
            "ReduceScatter",
            mybir.AluOpType.add,
            replica_groups=PAIRS,
            ins=[mstage.opt()],
            outs=[mpeer.opt()],
        )
        mp = []
        for dt_ in range(DT):
            t_ = m_pool.tile([P, SENDW], bf16, name=f"mp{dt_}", tag="m")
            nc.gpsimd.dma_start(out=t_[:], in_=mpeer[dt_])
            mp.append(t_)

        # out[t,e] = sum_d x[t,d] M[d,e], own-half rows. The own 640 cols
        # run first so the PE is busy while the M exchange is in flight.
        pieces = ((0, FREE, None), (FREE, OWN - FREE, None), (OWN, SENDW, mp))
        for off, w, src in pieces:
            for tt in range(TT):
                psum = ps_pool.tile([P, w], f32, name="pso", tag="ps")
                for dt_ in range(DT):
                    rhs = src[dt_][:] if src is not None else ms[dt_][:, off : off + w]
                    nc.tensor.matmul(
                        psum[:],
                        xts[dt_][:, ts(tt, P)],
                        rhs,
                        start=(dt_ == 0),
                        stop=(dt_ == DT - 1),
                    )
                o = ot_pool.tile([P, w], bf16, name="ot", tag="ot")
                if tt % 2 == 0:
                    nc.scalar.copy(o[:], psum[:])
                    nc.scalar.dma_start(out=out[ts(tt, P), off : off + w], in_=o[:])
                else:
                    nc.vector.tensor_copy(o[:], psum[:])
                    nc.sync.dma_start(out=out[ts(tt, P), off : off + w], in_=o[:])


def _build():
    _install_axon_ntff_shim()
    import concourse.mybir as mybir
    import concourse.tile as tile
    from concourse import bacc

    f32 = mybir.dt.float32
    bf16 = mybir.dt.bfloat16
    nc = bacc.Bacc("TRN2", target_bir_lowering=False, debug=False, num_devices=NCORES)
    xn = nc.dram_tensor("xn", [H, D], bf16, kind="ExternalInput").ap()
    xt = nc.dram_tensor("xt", [D, H], bf16, kind="ExternalInput").ap()
    wqT = nc.dram_tensor("wqT", [D, D], bf16, kind="ExternalInput").ap()
    wkT = nc.dram_tensor("wkT", [D, D], bf16, kind="ExternalInput").ap()
    wvT = nc.dram_tensor("wvT", [D, D], bf16, kind="ExternalInput").ap()
    woh = nc.dram_tensor("woh", [D, OWN], bf16, kind="ExternalInput").ap()
    mask = nc.dram_tensor("mask", [P, 2], f32, kind="ExternalInput").ap()
    out = nc.dram_tensor("out", [H, D], bf16, kind="ExternalOutput").ap()

    with tile.TileContext(nc) as tc:
        _trace_kernel(tc, xn, xt, wqT, wkT, wvT, woh, mask, out)
    nc.compile()
    return nc


def kernel(x, w_q, w_k, w_v, w_o):
    global LAST_RESULTS
    import ml_dtypes
    from concourse import bass_utils

    if "nc" not in _STATE:
        _STATE["nc"] = _build()
    nc = _STATE["nc"]

    bf16 = ml_dtypes.bfloat16
    x = np.ascontiguousarray(x, dtype=np.float32)
    wqT = np.asarray(w_q, dtype=np.float32).T.astype(bf16)
    wkT = np.asarray(w_k, dtype=np.float32).T.astype(bf16)
    wvT = np.asarray(w_v, dtype=np.float32).T.astype(bf16)
    wof = np.asarray(w_o, dtype=np.float32)

    in_maps = []
    for core in range(NCORES):
        b, half = core // 2, core % 2
        xh = x[b, half * H : (half + 1) * H]
        m = np.zeros((P, 2), dtype=np.float32)
        m[:, 1 - half] = 1.0  # zero own slot; pair position == half
        # "Rotated" col e == natural col (e + 512h) mod 1024: each core's C
        # window is a host-side roll+slice of wo's columns.
        woh = np.ascontiguousarray(
            np.roll(wof, -512 * half, axis=1)[:, :OWN]
        ).astype(bf16)
        in_maps.append(
            {
                "xn": xh.astype(bf16),
                "xt": xh.T.astype(bf16),
                "wqT": wqT,
                "wkT": wkT,
                "wvT": wvT,
                "woh": woh,
                "mask": m,
            }
        )

    LAST_RESULTS = bass_utils.run_bass_kernel_spmd(
        nc, in_maps, core_ids=list(range(NCORES))
    )
    out = np.empty((B, T, D), dtype=np.float32)
    for core in range(NCORES):
        b, half = core // 2, core % 2
        res = np.asarray(LAST_RESULTS.results[core]["out"], dtype=np.float32)
        rows = slice(half * H, (half + 1) * H)
        # un-rotate: natural col c lives at rotated col (c - 512h) mod 1024
        out[b, rows] = np.roll(res, 512 * half, axis=1)
    return out
